# revision 13
# baseline (speedup 1.0000x reference)
"""Trainium2 Bass kernel for nn_BinarizeLayer.

out[b, f] = (medians[f] > 0) AND (inputs[b, f] >= medians[f])

Host preprocessing folds the two conditions into one comparison:
m2[f] = medians[f] if medians[f] > 0 else +inf, so out = inputs >= m2
(inputs are finite, so x >= +inf is always False).

Data-parallel over 8 NeuronCores: each core handles a 2048-row slice of
the 16384x8192 f32 input. Per 128-row tile:
  - DVE compares against the median row (host sends 8 replicated rows,
    gpsimd SWDGE doubling DMAs replicate 8->128 partitions off the
    load queues), writing 0/1 bf16 bits in place over the head of the
    f32 tile (write trails read, so no hazard);
  - the tensor engine bit-packs 8 batch rows per byte with one constant
    [128,16] matmul weight (2^(p%8) block-diagonal), accumulating exact
    small integers in PSUM;
  - the scalar engine evacuates PSUM to SBUF with an f32->u8 cast, and
    each 2K-column chunk is stored as soon as it is ready.
Each core stores 2 MiB of packed bytes instead of 16 MiB; the host
unpacks bits (along the batch axis) back to the full bool array.

Engine budget per core: DVE ~139us, PE ~91us, ACT ~122us, all under the
~190us DMA time for 64.25 MiB in + 2 MiB out at ~358 GB/s per NC.
"""

import numpy as np

import concourse.bacc as bacc
import concourse.mybir as mybir
from concourse import tile
from concourse.bass_utils import run_bass_kernel_spmd

N_CORES = 8
B, F = 16384, 8192
BS = B // N_CORES  # rows per core
P = 128  # SBUF partitions
N_TILES = BS // P  # row-tiles per core
G = P // 8  # packed rows per tile (16)
MM_N = 512  # matmul free-dim chunk (one PSUM bank)
PS_W = 2048  # PSUM tile width (4 banks)
MR = 8  # replicated median rows sent from host


def _build():
    nc = bacc.Bacc(
        "TRN2",
        target_bir_lowering=False,
        debug=False,
        num_devices=N_CORES,
    )
    x = nc.declare_dram_parameter("x", [BS, F], mybir.dt.float32, isOutput=False)
    med = nc.declare_dram_parameter("med", [MR, F], mybir.dt.float32, isOutput=False)
    pw = nc.declare_dram_parameter("pw", [P, G], mybir.dt.float32, isOutput=False)
    out = nc.declare_dram_parameter(
        "out", [BS // 8, F], mybir.dt.uint8, isOutput=True
    )

    with tile.TileContext(nc) as tc:
        with (
            tc.tile_pool(name="const", bufs=1) as cpool,
            tc.tile_pool(name="xp", bufs=3) as xpool,
            tc.tile_pool(name="bp", bufs=2) as bpool,
            tc.tile_pool(name="op", bufs=2) as opool,
            tc.tile_pool(name="ps", bufs=2, space="PSUM") as pspool,
        ):
            # Median row to all 128 partitions: one small DRAM load plus
            # doubling SBUF->SBUF DMAs on the gpsimd (SWDGE) queues so
            # they never queue behind the big input loads.
            med_t = cpool.tile([P, F], mybir.dt.float32)
            nc.gpsimd.dma_start(out=med_t[0:MR, :], in_=med[:])
            k = MR
            while k < P:
                nc.gpsimd.dma_start(out=med_t[k : 2 * k, :], in_=med_t[0:k, :])
                k *= 2
            # Pack weights, cast to bf16 for the PE (values 2^k, exact).
            pw_f32 = cpool.tile([P, G], mybir.dt.float32)
            pw_t = cpool.tile([P, G], mybir.dt.bfloat16)
            nc.gpsimd.dma_start(out=pw_f32[:], in_=pw[:])
            nc.vector.tensor_copy(out=pw_t[:], in_=pw_f32[:])

            for i in range(N_TILES):
                xt = xpool.tile([P, F], mybir.dt.float32, tag="x")
                nc.sync.dma_start(out=xt[:], in_=x[i * P : (i + 1) * P, :])
                bt = bpool.tile([P, F], mybir.dt.bfloat16, tag="b")
                bits = bt[:, :]
                nc.vector.tensor_tensor(
                    bits, xt[:], med_t[:], mybir.AluOpType.is_ge
                )
                pk = opool.tile([G, F], mybir.dt.uint8, tag="o")
                for c in range(0, F, PS_W):
                    ps = pspool.tile([G, PS_W], mybir.dt.float32, tag="ps")
                    for n in range(0, PS_W, MM_N):
                        nc.tensor.matmul(
                            ps[:, n : n + MM_N],
                            pw_t[:],
                            bits[:, c + n : c + n + MM_N],
                            start=True,
                            stop=True,
                        )
                    nc.scalar.copy(out=pk[:, c : c + PS_W], in_=ps[:])
                    nc.sync.dma_start(
                        out=out[i * G : (i + 1) * G, c : c + PS_W],
                        in_=pk[:, c : c + PS_W],
                    )
    nc.compile()
    return nc


def _pack_weights():
    pw = np.zeros((P, G), dtype=np.float32)
    for p in range(P):
        pw[p, p // 8] = float(1 << (p % 8))
    return pw


def _in_maps(inputs, medians):
    x = np.ascontiguousarray(np.asarray(inputs, dtype=np.float32))
    m = np.asarray(medians, dtype=np.float32)
    m2 = np.where(m > 0, m, np.float32(np.inf)).astype(np.float32)
    med = np.ascontiguousarray(np.broadcast_to(m2[None, :], (MR, F)))
    pw = _pack_weights()
    return [
        {"x": x[c * BS : (c + 1) * BS], "med": med, "pw": pw}
        for c in range(N_CORES)
    ]


def kernel(inputs, medians):
    nc = _build()
    res = run_bass_kernel_spmd(nc, _in_maps(inputs, medians), list(range(N_CORES))).results
    out = np.concatenate(
        [np.unpackbits(r["out"], axis=0, bitorder="little") for r in res], axis=0
    )
    return out.astype(bool)


# revision 14
# speedup vs baseline: 1.0456x; 1.0456x over previous
"""Trainium2 Bass kernel for nn_BinarizeLayer.

out[b, f] = (medians[f] > 0) AND (inputs[b, f] >= medians[f])

Host preprocessing folds the two conditions into one comparison:
m2[f] = medians[f] if medians[f] > 0 else +inf, so out = inputs >= m2
(inputs are finite, so x >= +inf is always False).

Data-parallel over 8 NeuronCores: each core handles a 2048-row slice of
the 16384x8192 f32 input, processed as 64 [128 rows x 2048 cols] 1 MiB
chunks so DMA completions arrive at a fine grain and every stage
pipelines:
  - DVE compares the chunk against the (host-replicated) median rows,
    emitting 0/1 bf16 bits;
  - the tensor engine bit-packs 8 batch rows per byte with one constant
    [128,16] matmul weight (2^(p%8) block-diagonal), accumulating exact
    small integers in PSUM;
  - the scalar engine evacuates PSUM to SBUF with an f32->u8 cast and
    the chunk is stored immediately.
Each core stores 2 MiB of packed bytes instead of 16 MiB; the host
unpacks bits (along the batch axis) back to the full bool array.

Engine budget per core: DVE ~148us, PE ~95us, ACT ~132us, all under the
~196us DMA time for 68 MiB in + 2 MiB out at ~358 GB/s per NC.
"""

import numpy as np

import concourse.bacc as bacc
import concourse.mybir as mybir
from concourse import tile
from concourse.bass_utils import run_bass_kernel_spmd

N_CORES = 8
B, F = 16384, 8192
BS = B // N_CORES  # rows per core
P = 128  # SBUF partitions
N_TILES = BS // P  # row-tiles per core
G = P // 8  # packed rows per tile (16)
MM_N = 512  # matmul free-dim chunk (one PSUM bank)
W = 2048  # column-chunk width (1 MiB f32 loads)
N_COL = F // W


def _build():
    nc = bacc.Bacc(
        "TRN2",
        target_bir_lowering=False,
        debug=False,
        num_devices=N_CORES,
    )
    x = nc.declare_dram_parameter("x", [BS, F], mybir.dt.float32, isOutput=False)
    med = nc.declare_dram_parameter("med", [P, F], mybir.dt.float32, isOutput=False)
    pw = nc.declare_dram_parameter("pw", [P, G], mybir.dt.float32, isOutput=False)
    out = nc.declare_dram_parameter(
        "out", [BS // 8, F], mybir.dt.uint8, isOutput=True
    )

    with tile.TileContext(nc) as tc:
        with (
            tc.tile_pool(name="const", bufs=1) as cpool,
            tc.tile_pool(name="xp", bufs=8) as xpool,
            tc.tile_pool(name="bp", bufs=4) as bpool,
            tc.tile_pool(name="op", bufs=4) as opool,
            tc.tile_pool(name="ps", bufs=2, space="PSUM") as pspool,
        ):
            med_t = cpool.tile([P, F], mybir.dt.float32)
            nc.sync.dma_start(out=med_t[:], in_=med[:])
            # Pack weights, cast to bf16 for the PE (values 2^k, exact).
            pw_f32 = cpool.tile([P, G], mybir.dt.float32)
            pw_t = cpool.tile([P, G], mybir.dt.bfloat16)
            nc.sync.dma_start(out=pw_f32[:], in_=pw[:])
            nc.vector.tensor_copy(out=pw_t[:], in_=pw_f32[:])

            for i in range(N_TILES):
                rows = slice(i * P, (i + 1) * P)
                orows = slice(i * G, (i + 1) * G)
                for c in range(0, F, W):
                    xt = xpool.tile([P, W], mybir.dt.float32, tag="x")
                    nc.sync.dma_start(out=xt[:], in_=x[rows, c : c + W])
                    bt = bpool.tile([P, W], mybir.dt.bfloat16, tag="b")
                    nc.vector.tensor_tensor(
                        bt[:], xt[:], med_t[:, c : c + W], mybir.AluOpType.is_ge
                    )
                    ps = pspool.tile([G, W], mybir.dt.float32, tag="ps")
                    for n in range(0, W, MM_N):
                        nc.tensor.matmul(
                            ps[:, n : n + MM_N],
                            pw_t[:],
                            bt[:, n : n + MM_N],
                            start=True,
                            stop=True,
                        )
                    pk = opool.tile([G, W], mybir.dt.uint8, tag="o")
                    nc.scalar.copy(out=pk[:], in_=ps[:])
                    nc.sync.dma_start(out=out[orows, c : c + W], in_=pk[:])
    nc.compile()
    return nc


def _pack_weights():
    pw = np.zeros((P, G), dtype=np.float32)
    for p in range(P):
        pw[p, p // 8] = float(1 << (p % 8))
    return pw


def _in_maps(inputs, medians):
    x = np.ascontiguousarray(np.asarray(inputs, dtype=np.float32))
    m = np.asarray(medians, dtype=np.float32)
    m2 = np.where(m > 0, m, np.float32(np.inf)).astype(np.float32)
    med = np.ascontiguousarray(np.broadcast_to(m2[None, :], (P, F)))
    pw = _pack_weights()
    return [
        {"x": x[c * BS : (c + 1) * BS], "med": med, "pw": pw}
        for c in range(N_CORES)
    ]


def kernel(inputs, medians):
    nc = _build()
    res = run_bass_kernel_spmd(nc, _in_maps(inputs, medians), list(range(N_CORES))).results
    out = np.concatenate(
        [np.unpackbits(r["out"], axis=0, bitorder="little") for r in res], axis=0
    )
    return out.astype(bool)


# revision 15
# speedup vs baseline: 1.1034x; 1.0553x over previous
"""Trainium2 Bass kernel for nn_BinarizeLayer.

out[b, f] = (medians[f] > 0) AND (inputs[b, f] >= medians[f])

Host preprocessing folds the two conditions into one comparison:
m2[f] = medians[f] if medians[f] > 0 else +inf, so out = inputs >= m2
(inputs are finite, so x >= +inf is always False).

Data-parallel over 8 NeuronCores: each core handles a 2048-row slice of
the 16384x8192 f32 input, processed as 64 chunks of 32 consecutive rows
(1 MiB, fully contiguous in DRAM). The load's access pattern fans the
chunk onto 128 partitions: partition p holds quarter-row
(row 32i + p//4, cols (p%4)*2048 ..), so HBM reads stay sequential
while compute still uses all 128 lanes. Per chunk:
  - DVE compares against a median tile host-prepared in the same
    per-partition layout, emitting 0/1 bf16 bits;
  - the tensor engine bit-packs 8 partitions per byte with one constant
    [128,16] matmul weight (2^(p%8) block-diagonal), accumulating exact
    small integers in PSUM;
  - the scalar engine evacuates PSUM to SBUF with an f32->u8 cast and
    the 32 KiB packed chunk is stored contiguously.
Each core stores 2 MiB instead of 16 MiB; the host unpacks bits and
inverts the partition bijection with pure reshapes.

Engine budget per core: DVE ~150us, PE ~118us, ACT ~134us, all under
the ~185us DMA time for 65 MiB in + 2 MiB out at ~358 GB/s per NC.
"""

import numpy as np

import concourse.bacc as bacc
import concourse.mybir as mybir
from concourse import tile
from concourse.bass_utils import run_bass_kernel_spmd

N_CORES = 8
B, F = 16384, 8192
BS = B // N_CORES  # rows per core
P = 128  # SBUF partitions
R = 32  # rows per chunk
CQ = F // (P // R)  # columns per partition quarter-row (2048)
N_CHUNKS = BS // R  # chunks per core (64)
G = P // 8  # packed bytes' groups per chunk (16)
MM_N = 512  # matmul free-dim chunk (one PSUM bank)


def _build():
    nc = bacc.Bacc(
        "TRN2",
        target_bir_lowering=False,
        debug=False,
        num_devices=N_CORES,
    )
    x = nc.declare_dram_parameter("x", [BS, F], mybir.dt.float32, isOutput=False)
    med = nc.declare_dram_parameter("med", [P, CQ], mybir.dt.float32, isOutput=False)
    pw = nc.declare_dram_parameter("pw", [P, G], mybir.dt.float32, isOutput=False)
    out = nc.declare_dram_parameter(
        "out", [N_CHUNKS * G, CQ], mybir.dt.uint8, isOutput=True
    )
    # Chunk view: x as [N_CHUNKS, 128 partitions, 2048], partition
    # p = (row-in-chunk p//4, quarter p%4); DRAM order stays row-major,
    # so each chunk is one contiguous 1 MiB read.
    xv = x.rearrange("(i r) (c j) -> i (r c) j", r=R, c=P // R)

    with tile.TileContext(nc) as tc:
        with (
            tc.tile_pool(name="const", bufs=1) as cpool,
            tc.tile_pool(name="xp", bufs=10) as xpool,
            tc.tile_pool(name="bp", bufs=4) as bpool,
            tc.tile_pool(name="op", bufs=4) as opool,
            tc.tile_pool(name="ps", bufs=2, space="PSUM") as pspool,
        ):
            med_t = cpool.tile([P, CQ], mybir.dt.float32)
            nc.sync.dma_start(out=med_t[:], in_=med[:])
            # Pack weights, cast to bf16 for the PE (values 2^k, exact).
            pw_f32 = cpool.tile([P, G], mybir.dt.float32)
            pw_t = cpool.tile([P, G], mybir.dt.bfloat16)
            nc.sync.dma_start(out=pw_f32[:], in_=pw[:])
            nc.vector.tensor_copy(out=pw_t[:], in_=pw_f32[:])

            for i in range(N_CHUNKS):
                xt = xpool.tile([P, CQ], mybir.dt.float32, tag="x")
                nc.sync.dma_start(out=xt[:], in_=xv[i])
                bt = bpool.tile([P, CQ], mybir.dt.bfloat16, tag="b")
                nc.vector.tensor_tensor(
                    bt[:], xt[:], med_t[:], mybir.AluOpType.is_ge
                )
                ps = pspool.tile([G, CQ], mybir.dt.float32, tag="ps")
                for n in range(0, CQ, MM_N):
                    nc.tensor.matmul(
                        ps[:, n : n + MM_N],
                        pw_t[:],
                        bt[:, n : n + MM_N],
                        start=True,
                        stop=True,
                    )
                pk = opool.tile([G, CQ], mybir.dt.uint8, tag="o")
                nc.scalar.copy(out=pk[:], in_=ps[:])
                nc.sync.dma_start(out=out[i * G : (i + 1) * G, :], in_=pk[:])
    nc.compile()
    return nc


def _pack_weights():
    pw = np.zeros((P, G), dtype=np.float32)
    for p in range(P):
        pw[p, p // 8] = float(1 << (p % 8))
    return pw


def _in_maps(inputs, medians):
    x = np.ascontiguousarray(np.asarray(inputs, dtype=np.float32))
    m = np.asarray(medians, dtype=np.float32)
    m2 = np.where(m > 0, m, np.float32(np.inf)).astype(np.float32)
    # med_t[p, j] = m2[(p%4)*CQ + j], tiled for all 32 row-groups.
    med = np.ascontiguousarray(
        np.broadcast_to(m2.reshape(1, P // R, CQ), (R, P // R, CQ)).reshape(P, CQ)
    )
    pw = _pack_weights()
    return [
        {"x": x[c * BS : (c + 1) * BS], "med": med, "pw": pw}
        for c in range(N_CORES)
    ]


def _decode(packed):
    """[N_CHUNKS*G, CQ] u8 -> [BS, F] bool for one core."""
    a = packed.reshape(N_CHUNKS, G, 1, CQ)
    bits = np.unpackbits(a, axis=2, bitorder="little")  # [i, g, k, j]
    # partition p = 8g + k -> (row p//4, quarter p%4)
    bits = bits.reshape(N_CHUNKS, P, CQ).reshape(N_CHUNKS, R, P // R, CQ)
    return bits.reshape(BS, F)


def kernel(inputs, medians):
    nc = _build()
    res = run_bass_kernel_spmd(nc, _in_maps(inputs, medians), list(range(N_CORES))).results
    out = np.concatenate([_decode(r["out"]) for r in res], axis=0)
    return out.astype(bool)


# revision 16
# speedup vs baseline: 1.2423x; 1.1259x over previous
"""Trainium2 Bass kernel for nn_BinarizeLayer.

out[b, f] = (medians[f] > 0) AND (inputs[b, f] >= medians[f])

Host preprocessing folds the two conditions into one comparison:
m2[f] = medians[f] if medians[f] > 0 else +inf, so out = inputs >= m2
(inputs are finite, so x >= +inf is always False).

Data-parallel over 8 NeuronCores: each core handles a 2048-row slice of
the 16384x8192 f32 input, processed as 64 chunks of 32 consecutive rows
(1 MiB, fully contiguous in DRAM). The load's access pattern fans the
chunk onto 128 partitions: partition p holds quarter-row
(row 32i + p//4, cols (p%4)*2048 ..), so HBM reads stay sequential
while compute still uses all 128 lanes. Per chunk:
  - DVE compares against a median tile host-prepared in the same
    per-partition layout, emitting 0/1 bf16 bits;
  - the tensor engine bit-packs 8 partitions per byte with one constant
    [128,16] matmul weight (2^(p%8) block-diagonal), accumulating exact
    small integers in PSUM;
  - the scalar engine evacuates PSUM to SBUF with an f32->u8 cast and
    the 32 KiB packed chunk is stored contiguously.
Each core stores 2 MiB instead of 16 MiB; the host unpacks bits and
inverts the partition bijection with pure reshapes.

Engine budget per core: DVE ~150us, PE ~118us, ACT ~134us, all under
the ~185us DMA time for 65 MiB in + 2 MiB out at ~358 GB/s per NC.
"""

import numpy as np

import concourse.bacc as bacc
import concourse.mybir as mybir
from concourse import tile
from concourse.bass_utils import run_bass_kernel_spmd

N_CORES = 8
B, F = 16384, 8192
BS = B // N_CORES  # rows per core
P = 128  # SBUF partitions
R = 32  # rows per chunk
CQ = F // (P // R)  # columns per partition quarter-row (2048)
N_CHUNKS = BS // R  # chunks per core (64)
G = P // 8  # packed bytes' groups per chunk (16)
MM_N = 512  # matmul free-dim chunk (one PSUM bank)


def _build():
    nc = bacc.Bacc(
        "TRN2",
        target_bir_lowering=False,
        debug=False,
        num_devices=N_CORES,
    )
    x = nc.declare_dram_parameter("x", [BS, F], mybir.dt.float32, isOutput=False)
    med = nc.declare_dram_parameter("med", [P, CQ], mybir.dt.float32, isOutput=False)
    pw = nc.declare_dram_parameter("pw", [P, G], mybir.dt.float32, isOutput=False)
    out = nc.declare_dram_parameter(
        "out", [N_CHUNKS * G, CQ], mybir.dt.uint8, isOutput=True
    )
    # Chunk view: x as [N_CHUNKS, 128 partitions, 2048], partition
    # p = (row-in-chunk p//4, quarter p%4); DRAM order stays row-major,
    # so each chunk is one contiguous 1 MiB read.
    xv = x.rearrange("(i r) (c j) -> i (r c) j", r=R, c=P // R)

    with tile.TileContext(nc) as tc:
        with (
            tc.tile_pool(name="const", bufs=1) as cpool,
            tc.tile_pool(name="xp", bufs=10) as xpool,
            tc.tile_pool(name="bp", bufs=4) as bpool,
            tc.tile_pool(name="op", bufs=4) as opool,
            tc.tile_pool(name="ps", bufs=2, space="PSUM") as pspool,
        ):
            med_t = cpool.tile([P, CQ], mybir.dt.float32)
            nc.sync.dma_start(out=med_t[:], in_=med[:])
            # Pack weights, cast to bf16 for the PE (values 2^k, exact).
            pw_f32 = cpool.tile([P, G], mybir.dt.float32)
            pw_t = cpool.tile([P, G], mybir.dt.bfloat16)
            nc.sync.dma_start(out=pw_f32[:], in_=pw[:])
            nc.vector.tensor_copy(out=pw_t[:], in_=pw_f32[:])

            for i in range(N_CHUNKS):
                xt = xpool.tile([P, CQ], mybir.dt.float32, tag="x")
                nc.sync.dma_start(out=xt[:], in_=xv[i])
                bt = bpool.tile([P, CQ], mybir.dt.bfloat16, tag="b")
                nc.vector.tensor_tensor(
                    bt[:], xt[:], med_t[:], mybir.AluOpType.is_ge
                )
                ps = pspool.tile([G, CQ], mybir.dt.float32, tag="ps")
                for n in range(0, CQ, MM_N):
                    nc.tensor.matmul(
                        ps[:, n : n + MM_N],
                        pw_t[:],
                        bt[:, n : n + MM_N],
                        start=True,
                        stop=True,
                    )
                pk = opool.tile([G, CQ], mybir.dt.uint8, tag="o")
                nc.scalar.copy(out=pk[:], in_=ps[:])
                # Store from the scalar engine's HWDGE ring: its wait on
                # the copy above is already satisfied at dispatch, so it
                # never blocks the SP sequencer's load queue.
                nc.scalar.dma_start(out=out[i * G : (i + 1) * G, :], in_=pk[:])
    nc.compile()
    return nc


def _pack_weights():
    pw = np.zeros((P, G), dtype=np.float32)
    for p in range(P):
        pw[p, p // 8] = float(1 << (p % 8))
    return pw


def _in_maps(inputs, medians):
    x = np.ascontiguousarray(np.asarray(inputs, dtype=np.float32))
    m = np.asarray(medians, dtype=np.float32)
    m2 = np.where(m > 0, m, np.float32(np.inf)).astype(np.float32)
    # med_t[p, j] = m2[(p%4)*CQ + j], tiled for all 32 row-groups.
    med = np.ascontiguousarray(
        np.broadcast_to(m2.reshape(1, P // R, CQ), (R, P // R, CQ)).reshape(P, CQ)
    )
    pw = _pack_weights()
    return [
        {"x": x[c * BS : (c + 1) * BS], "med": med, "pw": pw}
        for c in range(N_CORES)
    ]


def _decode(packed):
    """[N_CHUNKS*G, CQ] u8 -> [BS, F] bool for one core."""
    a = packed.reshape(N_CHUNKS, G, 1, CQ)
    bits = np.unpackbits(a, axis=2, bitorder="little")  # [i, g, k, j]
    # partition p = 8g + k -> (row p//4, quarter p%4)
    bits = bits.reshape(N_CHUNKS, P, CQ).reshape(N_CHUNKS, R, P // R, CQ)
    return bits.reshape(BS, F)


def kernel(inputs, medians):
    nc = _build()
    res = run_bass_kernel_spmd(nc, _in_maps(inputs, medians), list(range(N_CORES))).results
    out = np.concatenate([_decode(r["out"]) for r in res], axis=0)
    return out.astype(bool)
